# revision 21
# baseline (speedup 1.0000x reference)
"""Causal single-head attention (b=4, n=2048, d=1024) on 8 trn2 cores.

Sharding: 2 cores per batch element. Each batch's 16 query blocks (128
rows) are assigned to its core pair so that every core processes one
q-block at each "capacity" in {2,4,...,16} key-blocks: even-parity
cores take even-index q-blocks (odd causal limit), odd-parity cores
take odd-index ones (even causal limit). Odd causal limits waste one
fully-masked 128-key block; total per-core key-block visits = 72
(vs 68 ideal) and the instruction stream is identical on all cores
(pure SPMD) — only the data (gathered q rows + mask) differs.

Per core: K^T/V/Q^T projections (PE), scores = Q^T·K per q-block,
masked softmax (DVE reduce + ACT exp), PE transpose of the weights,
AV accumulation, 1/rowsum folded into the PSUM->SBUF copyback.
The 1/sqrt(d) score scale (2^-5, exact) is folded into Q^T.
"""

import numpy as np

P = 128
B, N, D = 4, 2048, 1024
NCORES = 8
CAPS = (16, 14, 12, 10, 8, 6, 4, 2)  # key-block capacity per slot
NEG = -1.0e30

# Matmul compute dtype: "f32" (exact, 4 cyc/row) or "f32r" (full rate,
# TF32-ish hardware numerics).
MM_DT = "f32r"

_prog_cache = {}


def _split_multi_waits(nc, max_waits=1):
    """walrus in this container rejects more than one sem wait per
    instruction ("Too many sync wait commands"). After Tile scheduling,
    hoist extra waits onto same-engine nops inserted just before the
    instruction (same blocking semantics: engine queues are in-order)."""
    from concourse import mybir

    n = 0
    for fn in nc.m.functions:
        for bb in fn.blocks:
            out = []
            for ins in bb.instructions:
                si = ins.sync_info
                waits = list(si.on_wait) if si and si.on_wait else []
                if len(waits) > max_waits:
                    extra = waits[:-max_waits]
                    si.on_wait = waits[-max_waits:]
                    for j in range(0, len(extra), max_waits):
                        nop = mybir.InstNoOp(
                            name=f"waitsplit_{n}", ins=[], outs=[],
                            engine=ins.engine)
                        n += 1
                        nop.sync_info = mybir.SyncInfo(
                            on_wait=extra[j:j + max_waits], on_update=[])
                        out.append(nop)
                out.append(ins)
            bb.instructions[:] = out


def _build_program(mm_dt_name):
    import concourse.bass as bass
    import concourse.tile as tile
    from concourse import mybir
    from concourse.masks import make_identity

    f32 = mybir.dt.float32
    mmdt = f32 if mm_dt_name == "f32" else mybir.dt.float32r

    nc = bass.Bass("TRN2", target_bir_lowering=False, debug=False,
                   num_devices=NCORES, dynamic_dma_scratch_size=2048)

    xqT_d = nc.dram_tensor("xqT", [D, 8 * P], mmdt, kind="ExternalInput").ap()
    xkT_d = nc.dram_tensor("xkT", [D, N], mmdt, kind="ExternalInput").ap()
    wq_d = nc.dram_tensor("wq", [D, D], mmdt, kind="ExternalInput").ap()
    wk_d = nc.dram_tensor("wk", [D, D], mmdt, kind="ExternalInput").ap()
    wv_d = nc.dram_tensor("wv", [D, D], mmdt, kind="ExternalInput").ap()
    mask_d = nc.dram_tensor("mask", [P, 2 * P], f32, kind="ExternalInput").ap()
    out_d = nc.dram_tensor("out", [8 * P, D], f32, kind="ExternalOutput").ap()

    DC = D // P  # 8 contraction chunks
    xqT_r = xqT_d.rearrange("(dc p) q -> p dc q", p=P)
    xkT_r = xkT_d.rearrange("(dc p) k -> p dc k", p=P)
    wq_r = wq_d.rearrange("(dc p) e -> p dc e", p=P)
    wk_r = wk_d.rearrange("(dc p) e -> p dc e", p=P)
    wv_r = wv_d.rearrange("(dc p) e -> p dc e", p=P)

    with tile.TileContext(nc) as tc:
        import contextlib
        with contextlib.ExitStack() as ctx:
            cpool = ctx.enter_context(tc.tile_pool(name="cpool", bufs=1))
            qtp = ctx.enter_context(tc.tile_pool(name="qtp", bufs=1))
            ktp = ctx.enter_context(tc.tile_pool(name="ktp", bufs=1))
            vp = ctx.enter_context(tc.tile_pool(name="vp", bufs=1))

            ident_f = cpool.tile([P, P], f32, name="ident_f")
            make_identity(nc, ident_f)
            ident = cpool.tile([P, P], mmdt, name="ident")
            nc.vector.tensor_copy(ident[:], ident_f[:])
            mask_sb = cpool.tile([P, 2 * P], f32, name="mask_sb")
            nc.sync.dma_start(mask_sb[:], mask_d)

            QT = qtp.tile([P, DC, 8 * P], mmdt, name="QT")
            KT = ktp.tile([P, DC, N], mmdt, name="KT")
            V = vp.tile([P, N // P, D], mmdt, name="V")

            # ---- projections ----
            # Weights stream as four [P, 2, D] quarters (8KB/partition)
            # through 5 shared slots so the next phase's weights prefetch
            # into free slots while the current phase computes.
            with tc.tile_pool(name="wpool", bufs=10) as wpool, \
                 tc.tile_pool(name="xpool", bufs=2) as xpool, \
                 tc.tile_pool(name="ppj", bufs=4, space="PSUM") as ppj:

                def load_w(src_r, nm):
                    qs = []
                    for i in range(DC):
                        t = wpool.tile([P, 1, D], mmdt, tag="w",
                                       name=f"{nm}_q{i}")
                        nc.sync.dma_start(t[:], src_r[:, i:i + 1, :])
                        qs.append(t)
                    return qs

                # Q^T[e, q] = sum_d Wq[d, e] * xqT[d, q], scaled by 1/32
                # first x slice is DMA'd before the weights so the PE can
                # start as soon as the first weight quarters land
                xs0 = xpool.tile([P, DC, 256], mmdt, tag="xs", name="xs_q0")
                nc.sync.dma_start(xs0[:], xqT_r[:, :, 0:256])
                wq2 = load_w(wq_r, "wq")
                for qt in range(4):
                    if qt == 0:
                        xs = xs0
                    else:
                        xs = xpool.tile([P, DC, 256], mmdt, tag="xs",
                                        name="xs_q")
                        nc.sync.dma_start(
                            xs[:], xqT_r[:, :, qt * 256:(qt + 1) * 256])
                    dcs = list(range(DC)) if qt < 3 else \
                        list(range(DC - 1, -1, -1))
                    for ec in range(DC):
                        ps = ppj.tile([P, 512], f32, tag="pj", name="ps_q")
                        for i, dc in enumerate(dcs):
                            nc.tensor.matmul(
                                ps[:, :256],
                                wq2[dc][:, 0, ec * P:(ec + 1) * P],
                                xs[:, dc, :],
                                start=(i == 0), stop=(i == DC - 1))
                        nc.vector.tensor_scalar_mul(
                            QT[:, ec, qt * 256:(qt + 1) * 256],
                            ps[:, :256], 1.0 / 32.0)

                # K^T[e, k] = sum_d Wk[d, e] * xkT[d, k]
                wk2 = load_w(wk_r, "wk")
                for kt in range(8):
                    xs = xpool.tile([P, DC, 256], mmdt, tag="xs", name="xs_k")
                    nc.sync.dma_start(xs[:], xkT_r[:, :, kt * 256:(kt + 1) * 256])
                    dcs = list(range(DC)) if kt < 7 else \
                        list(range(DC - 1, -1, -1))
                    for ec in range(DC):
                        ps = ppj.tile([P, 512], f32, tag="pj", name="ps_k")
                        for i, dc in enumerate(dcs):
                            nc.tensor.matmul(
                                ps[:, :256],
                                wk2[dc][:, 0, ec * P:(ec + 1) * P],
                                xs[:, dc, :],
                                start=(i == 0), stop=(i == DC - 1))
                        nc.vector.tensor_copy(
                            KT[:, ec, kt * 256:(kt + 1) * 256], ps[:, :256])

                # V[k, e] = sum_d xkT[d, k] * Wv[d, e]
                wv2 = load_w(wv_r, "wv")
                for kp in range(N // 256):
                    xs = xpool.tile([P, DC, 256], mmdt, tag="xs", name="xs_v")
                    nc.sync.dma_start(xs[:], xkT_r[:, :, kp * 256:(kp + 1) * 256])
                    for half in range(2):
                        kc = 2 * kp + half
                        for h in range(2):
                            ps = ppj.tile([P, 512], f32, tag="pj", name="ps_v")
                            for dc in range(DC):
                                nc.tensor.matmul(
                                    ps,
                                    xs[:, dc, half * P:(half + 1) * P],
                                    wv2[dc][:, 0, h * 512:(h + 1) * 512],
                                    start=(dc == 0), stop=(dc == DC - 1))
                            nc.vector.tensor_copy(
                                V[:, kc, h * 512:(h + 1) * 512], ps)

            # ---- attention, software-pipelined over the 8 slots ----
            with tc.tile_pool(name="scp", bufs=3) as scp, \
                 tc.tile_pool(name="wtp", bufs=2) as wtp, \
                 tc.tile_pool(name="obp", bufs=2) as obp, \
                 tc.tile_pool(name="stp", bufs=3) as stp, \
                 tc.tile_pool(name="psc", bufs=3, space="PSUM") as psc, \
                 tc.tile_pool(name="pav", bufs=3, space="PSUM") as pav, \
                 tc.tile_pool(name="ptr", bufs=2, space="PSUM") as ptr:

                scores = [None] * len(CAPS)
                stats = [None] * len(CAPS)

                def emit_scores(slot):
                    s = CAPS[slot]
                    L = P * s
                    sc = scp.tile([P, N], mmdt, tag="sc", name=f"sc{slot}")
                    st = stp.tile([P, 4], f32, tag="st", name=f"st{slot}")
                    scores[slot] = sc
                    stats[slot] = st
                    off = 0
                    widths = [512] * (L // 512) + ([256] if L % 512 else [])
                    for w in widths:
                        ps = psc.tile([P, 512], f32, tag="psc", name=f"pssc{slot}")
                        for ec in range(DC):
                            nc.tensor.matmul(
                                ps[:, :w],
                                QT[:, ec, slot * P:(slot + 1) * P],
                                KT[:, ec, off:off + w],
                                start=(ec == 0), stop=(ec == DC - 1))
                        end = off + w
                        if end == L:
                            if w == 512:
                                nc.vector.tensor_copy(
                                    sc[:, off:off + 256], ps[:, 0:256])
                            nc.vector.tensor_add(
                                sc[:, L - 256:L], ps[:, w - 256:w], mask_sb[:])
                        else:
                            nc.vector.tensor_copy(sc[:, off:end], ps[:, :w])
                        off = end
                    # softmax stats + in-place exp
                    nc.vector.tensor_reduce(
                        st[:, 0:1], sc[:, :L], axis=mybir.AxisListType.X,
                        op=mybir.AluOpType.max, negate=True)
                    nc.scalar.activation(
                        sc[:, :L], sc[:, :L], mybir.ActivationFunctionType.Exp,
                        bias=st[:, 0:1], scale=1.0, accum_out=st[:, 1:2])
                    nc.vector.reciprocal(st[:, 2:3], st[:, 1:2])

                def emit_av(slot):
                    s = CAPS[slot]
                    sc = scores[slot]
                    st = stats[slot]
                    wt = wtp.tile([P, N // P, P], mmdt, tag="wt", name=f"wt{slot}")
                    for j in range(s):
                        pt = ptr.tile([P, P], mmdt, tag="ptr", name=f"pt{slot}")
                        nc.tensor.transpose(pt, sc[:, j * P:(j + 1) * P], ident)
                        nc.vector.tensor_copy(wt[:, j, :], pt)
                    avs = []
                    for h in range(2):
                        av = pav.tile([P, 512], f32, tag="pav", name=f"av{slot}_{h}")
                        avs.append(av)
                    for j in range(s):
                        for h in range(2):
                            nc.tensor.matmul(
                                avs[h],
                                wt[:, j, :],
                                V[:, j, h * 512:(h + 1) * 512],
                                start=(j == 0), stop=(j == s - 1))
                    ob = obp.tile([P, D], f32, tag="ob", name=f"ob{slot}")
                    for h in range(2):
                        nc.vector.tensor_scalar_mul(
                            ob[:, h * 512:(h + 1) * 512], avs[h], st[:, 2:3])
                    nc.sync.dma_start(out_d[slot * P:(slot + 1) * P, :], ob)

                emit_scores(0)
                emit_scores(1)
                for b_ in range(len(CAPS)):
                    if b_ + 2 < len(CAPS):
                        emit_scores(b_ + 2)
                    emit_av(b_)

    _split_multi_waits(nc)
    return nc


def _host_prep(x, Wq, Wk, Wv):
    """Build per-core input maps."""
    x = np.ascontiguousarray(x, dtype=np.float32)
    tri = np.where(
        np.arange(P)[None, :] <= np.arange(P)[:, None], 0.0, NEG
    ).astype(np.float32)
    mask_even = np.concatenate(  # parity 0: diag block then fully-masked block
        [tri, np.full((P, P), NEG, np.float32)], axis=1)
    mask_odd = np.concatenate(  # parity 1: fully-visible block then diag block
        [np.zeros((P, P), np.float32), tri], axis=1)

    in_maps = []
    for c in range(NCORES):
        bi, r = c // 2, c % 2
        rbs = [s - 2 + r for s in CAPS]
        xq = np.concatenate([x[bi, rb * P:(rb + 1) * P, :] for rb in rbs], axis=0)
        in_maps.append({
            "xqT": np.ascontiguousarray(xq.T),
            "xkT": np.ascontiguousarray(x[bi].T),
            "wq": np.ascontiguousarray(Wq, dtype=np.float32),
            "wk": np.ascontiguousarray(Wk, dtype=np.float32),
            "wv": np.ascontiguousarray(Wv, dtype=np.float32),
            "mask": mask_odd if r else mask_even,
        })
    return in_maps


def _host_gather(results):
    out = np.empty((B, N, D), dtype=np.float32)
    for c in range(NCORES):
        bi, r = c // 2, c % 2
        res = results[c]["out"]
        for k, s in enumerate(CAPS):
            rb = s - 2 + r
            out[bi, rb * P:(rb + 1) * P, :] = res[k * P:(k + 1) * P, :]
    return out


def kernel(x, Wq, Wk, Wv, _trace=False, _trace_kwargs=None):
    from concourse.bass_utils import run_bass_kernel_spmd

    key = MM_DT
    if key not in _prog_cache:
        _prog_cache[key] = _build_program(key)
    nc = _prog_cache[key]

    in_maps = _host_prep(x, Wq, Wk, Wv)
    kw = dict(_trace_kwargs or {})
    res = run_bass_kernel_spmd(nc, in_maps, list(range(NCORES)),
                               trace=_trace, **kw)
    out = _host_gather(res.results)
    if _trace:
        return out, res
    return out
